# revision 66
# baseline (speedup 1.0000x reference)
"""Trainium2 Bass kernel for nn_BertMoEClassifier.

Full-input contract: kernel(**inputs) takes the unsharded numpy inputs and
returns the full [32, 512, 2] logits.  Internally: data-parallel over the
batch dim across 8 NeuronCores (4 batches = 2048 tokens per core), dense
8-expert MoE, no collectives.

Shapes (hardcoded): B=32 S=512 C=3072 D=768 H=1024 E=8 K=2 L=2.

Numerics:
  proj: 3-term product tuned to the PE's f32r operand truncation (11
    explicit mantissa bits): hi11(x) @ hi11(w) runs exact in one f32r
    pass; the two first-order residual terms run as fp8 DoubleRow
    matmuls at 4x throughput (their 2^-17-scale makes fp8 error
    negligible).  Router sees ~2e-6 logit error -> top-2 matches fp32.
  experts: fp8(e4m3) DoubleRow for both MLP matmuls (w1*64, w2*64,
    seq*16 scaling keeps values out of fp8 denormals), fp32 PSUM.
    Combine weights fold in via per-partition DVE scalars.
"""

from contextlib import ExitStack

import ml_dtypes
import numpy as np

import concourse.bacc as bacc
import concourse.bass as bass
import concourse.mybir as mybir
from concourse import bass_utils
import concourse.tile as tile
from concourse.masks import make_identity

F32 = mybir.dt.float32
F32R = mybir.dt.float32r
F16 = mybir.dt.float16
FP8 = mybir.dt.float8e4
I32 = mybir.dt.int32
RSQRT_MAGIC = float(np.uint32(0x5F3759DF).view(np.float32))
AF = mybir.ActivationFunctionType
OP = mybir.AluOpType
DR = mybir.MatmulPerfMode.DoubleRow

B, S, C, D, H, E, L = 32, 512, 3072, 768, 1024, 8, 2
NCORES = 8
T = (B // NCORES) * S            # 2048 tokens per core
NT = T // 128                    # 16 token tiles
KC = C // 128                    # 24 contraction chunks (proj)
KP = C // 256                    # 12 paired chunks (fp8 DoubleRow)
KD = D // 128                    # 6 chunks of D
KH = H // 128                    # 8 chunks of H
NEG_BIG = -1.0e30
EPS = 1e-5

# fp8 piece scales (powers of two; folded out via DVE at psum-combine time)
SA = 2.0 ** 12        # x residual piece
SW8 = 2.0 ** 5        # fp8(w) piece
SWB = 2.0 ** 17       # w residual piece
RESID_SCALE = 1.0 / (SA * SW8)    # == 1/SWB; both resid products share it
SSEQ = 16.0           # seq -> fp8 scale
SW1 = 64.0            # w1 -> fp8 scale
SW2 = 64.0            # w2 -> fp8 scale
MOE_PSUM_SCALE = SSEQ * SW1       # mm1 psum carries SSEQ*SW1 * (x@w1)

_CACHE = {}
FLAGS_DEFAULT = dict(ln1_id=False, ln2_id=False, pb_zero=False,
                     b1_zero=False, b2_zero=False, cb_zero=False)
import os
PHASES = os.environ.get("K_PHASES", "123")


def _bcast_row(h_ap, off, n):
    """AP broadcasting a DRAM row of n elements across 128 partitions."""
    return bass.AP(tensor=h_ap.tensor, offset=h_ap.offset + off, ap=[[0, 128], [1, n]])


def _build(flags):
    nc = bacc.Bacc("TRN2", target_bir_lowering=False, debug=False)

    # tile-major x-side inputs: one big contiguous line per (tile, partition)
    hh_d = nc.dram_tensor("hh", [NT, 128, KC, 128], F32, kind="ExternalInput")
    xq_d = nc.dram_tensor("xq", [NT, 128, KP, 2, 2, 128], FP8, kind="ExternalInput")
    pwh_d = nc.dram_tensor("pwh", [C, D], F32, kind="ExternalInput")
    pw8_d = nc.dram_tensor("pw8", [KP, 128, 2, D], FP8, kind="ExternalInput")
    pwb_d = nc.dram_tensor("pwb", [KP, 128, 2, D], FP8, kind="ExternalInput")
    pb_d = nc.dram_tensor("pb", [D], F32, kind="ExternalInput")
    g1_d = nc.dram_tensor("g1", [D], F32, kind="ExternalInput")
    be1_d = nc.dram_tensor("be1", [D], F32, kind="ExternalInput")
    g2_d = nc.dram_tensor("g2", [D], F32, kind="ExternalInput")
    be2_d = nc.dram_tensor("be2", [D], F32, kind="ExternalInput")
    gw_d = nc.dram_tensor("gw", [128, KD, E], F32, kind="ExternalInput")
    gb_d = nc.dram_tensor("gb", [E], F32, kind="ExternalInput")
    w1q_d = nc.dram_tensor("w1q", [E, 3, 128, 2, H], FP8, kind="ExternalInput")
    b1_d = nc.dram_tensor("b1", [128, E, KH], F32, kind="ExternalInput")
    w2q_d = nc.dram_tensor("w2q", [E, 4, 128, 2, D], FP8, kind="ExternalInput")
    b2_d = nc.dram_tensor("b2", [E, D], F32, kind="ExternalInput")
    cwj_d = nc.dram_tensor("cwj", [128, KD, L], F32, kind="ExternalInput")
    csum_d = nc.dram_tensor("csum", [L], F32, kind="ExternalInput")
    cb_d = nc.dram_tensor("cb", [L], F32, kind="ExternalInput")
    out_d = nc.dram_tensor("out", [T, L], F32, kind="ExternalOutput")

    with ExitStack() as ctx:
        tc = ctx.enter_context(tile.TileContext(nc))
        persist = ctx.enter_context(tc.tile_pool(name="persist", bufs=1))

        # ---- persistent tiles -------------------------------------------
        acc16 = [persist.tile([128, D], F16, name=f"acc{t}", tag=f"acc{t}")
                 for t in range(NT)]
        seq8 = [persist.tile([128, 2, T], FP8, name=f"seq8_{k}", tag=f"seq8_{k}")
                for k in range(3)]
        comb = [persist.tile([128, E], F32, name=f"comb{t}", tag=f"comb{t}")
                for t in range(NT)]
        ident = persist.tile([128, 128], F32, name="ident", tag="ident")
        gwsb = persist.tile([128, KD, E], F32, name="gwsb", tag="gwsb")
        gbb = persist.tile([128, E], F32, name="gbb", tag="gbb")
        b1sb = None
        if not flags["b1_zero"]:
            b1sb = persist.tile([128, E, KH], F32, name="b1sb", tag="b1sb")
        epst = persist.tile([128, 1], F32, name="epst", tag="epst")
        cmagic = persist.tile([128, 1], F32, name="cmagic", tag="cmagic")
        nc.vector.memset(cmagic, RSQRT_MAGIC)
        # expert-0 MoE weights, preloaded during phase 1's DMA slack so the
        # phase-2 weight stream gets a one-expert head start
        w1pre = [persist.tile([128, 2, H], FP8, name=f"w1pre{kd}",
                              tag=f"w1pre{kd}") for kd in range(3)]
        w2pre = [persist.tile([128, 2, D], FP8, name=f"w2pre{k}",
                              tag=f"w2pre{k}") for k in range(4)]

        def newton_rsqrt(pool, tag, mvvar, iters):
            """rstd = 1/sqrt(var+eps) on DVE only (no ACT table)."""
            v = pool.tile([128, 1], F32, name=f"v{tag}", tag=f"v{tag}")
            nc.vector.tensor_tensor(out=v, in0=mvvar, in1=epst, op=OP.add)
            ih = pool.tile([128, 1], I32, name=f"ih{tag}", tag=f"ih{tag}")
            nc.vector.tensor_scalar(out=ih, in0=v.bitcast(I32), scalar1=1,
                                    scalar2=None, op0=OP.arith_shift_right)
            yi = pool.tile([128, 1], I32, name=f"yi{tag}", tag=f"yi{tag}")
            nc.vector.tensor_tensor(out=yi, in0=cmagic.bitcast(I32), in1=ih,
                                    op=OP.subtract)
            y = yi.bitcast(F32)
            for i in range(iters):
                a = pool.tile([128, 1], F32, name=f"a{tag}_{i}", tag=f"a{tag}")
                nc.vector.tensor_tensor(out=a, in0=v, in1=y, op=OP.mult)
                nc.vector.tensor_tensor(out=a, in0=a, in1=y, op=OP.mult)
                nc.vector.tensor_scalar(out=a, in0=a, scalar1=-0.5,
                                        scalar2=1.5, op0=OP.mult, op1=OP.add)
                yn = pool.tile([128, 1], F32, name=f"yn{tag}_{i}",
                               tag=f"yn{tag}")
                nc.vector.tensor_tensor(out=yn, in0=y, in1=a, op=OP.mult)
                y = yn
            return y
        pbb = g1b = be1b = None
        if not flags["pb_zero"]:
            pbb = persist.tile([128, D], F32, name="pbb", tag="pbb")
            nc.sync.dma_start(out=pbb, in_=_bcast_row(pb_d.ap(), 0, D))
        if not flags["ln1_id"]:
            g1b = persist.tile([128, D], F32, name="g1b", tag="g1b")
            be1b = persist.tile([128, D], F32, name="be1b", tag="be1b")
            nc.sync.dma_start(out=g1b, in_=_bcast_row(g1_d.ap(), 0, D))
            nc.sync.dma_start(out=be1b, in_=_bcast_row(be1_d.ap(), 0, D))

        nc.sync.dma_start(out=gwsb, in_=gw_d.ap())
        nc.sync.dma_start(out=gbb, in_=_bcast_row(gb_d.ap(), 0, E))
        if b1sb is not None:
            nc.sync.dma_start(out=b1sb, in_=b1_d.ap())
        nc.vector.memset(epst, EPS)
        make_identity(nc, ident)

        # ============ Phase 1: proj + LN1 + GELU + router ================
        # Weight-stationary main pass: 3 groups of 8 contraction chunks;
        # each group streams all 16 token tiles, partials accumulate in
        # SBUF f32.  PE starts ~2us in (vs waiting for the full weight set).
        with tc.tile_pool(name="p1pw", bufs=12) as pwpool, \
             tc.tile_pool(name="p1pw8", bufs=1) as pw8pool, \
             tc.tile_pool(name="p1hs", bufs=3) as hspool, \
             tc.tile_pool(name="p1x8", bufs=2) as x8pool, \
             tc.tile_pool(name="p1acc", bufs=1) as accpool, \
             tc.tile_pool(name="p1sm", bufs=4) as smpool, \
             tc.tile_pool(name="p1st", bufs=2) as stgpool, \
             tc.tile_pool(name="p1v", bufs=4) as vpool, \
             tc.tile_pool(name="p1psA", bufs=2, space="PSUM") as psA, \
             tc.tile_pool(name="p1psR", bufs=1, space="PSUM") as psR, \
             tc.tile_pool(name="p1psT", bufs=2, space="PSUM") as psT:

            accf = [accpool.tile([128, D], F32, name=f"accf{t}", tag=f"accf{t}")
                    for t in range(NT)]
            pw8 = []
            pwb = []

            hs00 = None
            if "1" in PHASES:
                hs00 = hspool.tile([128, 8, 128], F32R, name="hs00", tag="hhs")
                nc.sync.dma_start(out=hs00,
                                  in_=hh_d.ap()[0, :, 0:8, :].bitcast(F32R))
            for cg in (range(3) if "1" in PHASES else []):
                pwg = []
                for ci in range(8):
                    c = cg * 8 + ci
                    w = pwpool.tile([128, D], F32R, name=f"pwh{c}", tag="pwh")
                    nc.sync.dma_start(
                        out=w,
                        in_=pwh_d.ap()[c * 128:(c + 1) * 128, :].bitcast(F32R))
                    pwg.append(w)
                for t in range(NT):
                    if cg == 0 and t == 0:
                        hhs = hs00
                    else:
                        hhs = hspool.tile([128, 8, 128], F32R,
                                          name=f"hs{cg}_{t}", tag="hhs")
                        nc.sync.dma_start(
                            out=hhs,
                            in_=hh_d.ap()[t, :, cg * 8:(cg + 1) * 8, :].bitcast(F32R))
                    if cg == 1 and t < KP:
                        w8 = pw8pool.tile([128, 2, D], FP8, name=f"pw8_{t}",
                                          tag=f"pw8_{t}")
                        nc.sync.dma_start(out=w8, in_=pw8_d.ap()[t])
                        pw8.append(w8)
                    if cg == 2 and t < KP:
                        wb = pw8pool.tile([128, 2, D], FP8, name=f"pwb_{t}",
                                          tag=f"pwb_{t}")
                        nc.sync.dma_start(out=wb, in_=pwb_d.ap()[t])
                        pwb.append(wb)
                    ps = psA.tile([128, D], F32, name=f"pm{cg}_{t}", tag="paM")
                    for ci in range(8):
                        nc.tensor.matmul(ps[:, 0:512], hhs[:, ci, :],
                                         pwg[ci][:, 0:512],
                                         start=(ci == 0), stop=(ci == 7))
                        nc.tensor.matmul(ps[:, 512:768], hhs[:, ci, :],
                                         pwg[ci][:, 512:768],
                                         start=(ci == 0), stop=(ci == 7))
                    if cg == 0:
                        nc.vector.tensor_copy(out=accf[t], in_=ps)
                    else:
                        nc.vector.tensor_tensor(out=accf[t], in0=accf[t],
                                                in1=ps, op=OP.add)

            def chains(t):
                """DMA the tile's fp8 pieces and run the resid DR chains."""
                if t == 0:
                    xqt = xq0
                elif t == 1:
                    xqt = xq1
                else:
                    xqt = x8pool.tile([128, KP, 2, 2, 128], FP8, name=f"xq{t}",
                                      tag="xq")
                    nc.sync.dma_start(out=xqt, in_=xq_d.ap()[t])
                paR = psR.tile([128, D], F32, name=f"paR{t}", tag="paR")
                for c in range(KP):
                    nc.tensor.matmul(paR[:, 0:512], xqt[:, c, 0], pw8[c][:, :, 0:512],
                                     start=(c == 0), stop=False, perf_mode=DR)
                    nc.tensor.matmul(paR[:, 512:768], xqt[:, c, 0],
                                     pw8[c][:, :, 512:768],
                                     start=(c == 0), stop=False, perf_mode=DR)
                for c in range(KP):
                    nc.tensor.matmul(paR[:, 0:512], xqt[:, c, 1], pwb[c][:, :, 0:512],
                                     start=False, stop=(c == KP - 1), perf_mode=DR)
                    nc.tensor.matmul(paR[:, 512:768], xqt[:, c, 1],
                                     pwb[c][:, :, 512:768],
                                     start=False, stop=(c == KP - 1), perf_mode=DR)
                return paR

            def finish(t, paR):
                ts, te = t * 128, (t + 1) * 128
                x = accf[t]
                nc.vector.scalar_tensor_tensor(
                    out=x, in0=paR, scalar=RESID_SCALE, in1=x,
                    op0=OP.mult, op1=OP.add)
                if pbb is not None:
                    nc.vector.tensor_tensor(out=x, in0=x, in1=pbb, op=OP.add)

                # LN1 + GELU
                stats = smpool.tile([128, 3, 6], F32, name=f"st{t}", tag="stats")
                for sg in range(3):
                    nc.vector.bn_stats(out=stats[:, sg, :],
                                       in_=x[:, sg * 256:(sg + 1) * 256])
                mv = smpool.tile([128, 2], F32, name=f"mv{t}", tag="mv")
                nc.vector.bn_aggr(out=mv, in_=stats)
                rstd = newton_rsqrt(smpool, "L1", mv[:, 1:2], 3)
                if flags["ln1_id"]:
                    # fused LN+GELU: gelu(rstd*x - mu*rstd) in one ACT op
                    nb = smpool.tile([128, 1], F32, name=f"nb{t}", tag="nb")
                    nc.vector.tensor_scalar(out=nb, in0=mv[:, 0:1],
                                            scalar1=rstd, scalar2=-1.0,
                                            op0=OP.mult, op1=OP.mult)
                    nc.scalar.activation(out=x, in_=x, func=AF.Gelu,
                                         bias=nb, scale=rstd)
                else:
                    nc.vector.tensor_scalar(out=x, in0=x, scalar1=mv[:, 0:1],
                                            scalar2=rstd, op0=OP.subtract,
                                            op1=OP.mult)
                    nc.vector.tensor_tensor(out=x, in0=x, in1=g1b, op=OP.mult)
                    nc.vector.tensor_tensor(out=x, in0=x, in1=be1b, op=OP.add)
                    nc.scalar.activation(out=x, in_=x, func=AF.Gelu)

                # fp16 copy for the residual path
                nc.scalar.copy(out=acc16[t], in_=x)

                # transpose -> stg (router stationary + seq8 source)
                stg = stgpool.tile([128, KD, 128], F32, name=f"stg{t}", tag="stg")
                for j in range(KD):
                    pt = psT.tile([128, 128], F32, name=f"pt{t}_{j}", tag="psT")
                    nc.tensor.transpose(pt, x[:, j * 128:(j + 1) * 128], ident)
                    nc.scalar.copy(out=stg[:, j, :], in_=pt)
                    nc.scalar.mul(out=seq8[j // 2][:, j % 2, ts:te],
                                  in_=stg[:, j, :], mul=SSEQ)

                # router: gate matmul + softmax + top-2 renorm
                pr = psT.tile([128, E], F32, name=f"pr{t}", tag="psT")
                for j in range(KD):
                    nc.tensor.matmul(pr, stg[:, j, :], gwsb[:, j, :],
                                     start=(j == 0), stop=(j == KD - 1))
                lg = vpool.tile([128, E], F32, name=f"lg{t}", tag="lg")
                nc.vector.tensor_tensor(out=lg, in0=pr, in1=gbb, op=OP.add)
                m1 = smpool.tile([128, 1], F32, name=f"m1{t}", tag="m1")
                nc.vector.reduce_max(out=m1, in_=lg, axis=mybir.AxisListType.X)
                mm = vpool.tile([128, E], F32, name=f"mm{t}", tag="mm")
                nc.vector.tensor_scalar(out=mm, in0=lg, scalar1=m1,
                                        scalar2=None, op0=OP.is_ge)
                lg2 = vpool.tile([128, E], F32, name=f"lg2{t}", tag="lg2")
                nc.vector.scalar_tensor_tensor(out=lg2, in0=mm, scalar=NEG_BIG,
                                               in1=lg, op0=OP.mult, op1=OP.add)
                m2 = smpool.tile([128, 1], F32, name=f"m2{t}", tag="m2")
                nc.vector.reduce_max(out=m2, in_=lg2, axis=mybir.AxisListType.X)
                mk2 = vpool.tile([128, E], F32, name=f"mk2{t}", tag="mk2")
                nc.vector.tensor_scalar(out=mk2, in0=lg, scalar1=m2,
                                        scalar2=None, op0=OP.is_ge)
                # top-2 renormalized softmax == sigmoid of the logit gap:
                # w_top1 = sig(m1-m2) = (1+tanh((m1-m2)/2))/2  (tanh shares
                # the GELU activation table -> no table reload)
                diff = smpool.tile([128, 1], F32, name=f"df{t}", tag="df")
                nc.vector.tensor_tensor(out=diff, in0=m1, in1=m2,
                                        op=OP.subtract)
                th = smpool.tile([128, 1], F32, name=f"th{t}", tag="th")
                nc.scalar.activation(out=th, in_=diff, func=AF.Tanh, scale=0.5)
                w1s = smpool.tile([128, 1], F32, name=f"w1s{t}", tag="w1s")
                nc.vector.tensor_scalar(out=w1s, in0=th, scalar1=1.0,
                                        scalar2=0.5 / SW2, op0=OP.add,
                                        op1=OP.mult)
                w2s = smpool.tile([128, 1], F32, name=f"w2s{t}", tag="w2s")
                nc.vector.tensor_scalar(out=w2s, in0=th, scalar1=1.0,
                                        scalar2=-0.5 / SW2, op0=OP.subtract,
                                        op1=OP.mult)
                d2 = vpool.tile([128, E], F32, name=f"d2{t}", tag="d2")
                nc.vector.tensor_tensor(out=d2, in0=mk2, in1=mm,
                                        op=OP.subtract)
                t1 = vpool.tile([128, E], F32, name=f"t1{t}", tag="t1")
                nc.vector.tensor_scalar_mul(out=t1, in0=mm, scalar1=w1s)
                nc.vector.tensor_scalar_mul(out=d2, in0=d2, scalar1=w2s)
                nc.vector.tensor_tensor(out=comb[t], in0=t1, in1=d2, op=OP.add)

            # resid-loop head: prefetch the first two tiles' fp8 pieces
            # ahead of the expert-0 weight preload
            xq0 = x8pool.tile([128, KP, 2, 2, 128], FP8, name="xq00", tag="xq")
            nc.sync.dma_start(out=xq0, in_=xq_d.ap()[0])
            xq1 = x8pool.tile([128, KP, 2, 2, 128], FP8, name="xq01", tag="xq")
            nc.sync.dma_start(out=xq1, in_=xq_d.ap()[1])

            # expert-0 weight preload rides the resid loop's DMA slack
            for kd in range(3):
                nc.sync.dma_start(out=w1pre[kd], in_=w1q_d.ap()[0, kd])
            for k in range(4):
                nc.sync.dma_start(out=w2pre[k], in_=w2q_d.ap()[0, k])

            prev = None
            for t in (range(NT) if "1" in PHASES else []):
                cur = chains(t)
                if prev is not None:
                    finish(prev[0], prev[1])
                prev = (t, cur)
            if prev is not None:
                finish(prev[0], prev[1])

        # ============ Phase 2: dense MoE (fp8 DR) + inline LN2/cls =======
        with tc.tile_pool(name="p2w1", bufs=1) as w1pool, \
             tc.tile_pool(name="p2w2", bufs=1) as w2pool, \
             tc.tile_pool(name="p2h", bufs=7) as hpool, \
             tc.tile_pool(name="p3", bufs=2) as p3pool, \
             tc.tile_pool(name="p3c", bufs=1) as cpool, \
             tc.tile_pool(name="p3sm", bufs=4) as sm3, \
             tc.tile_pool(name="p3out", bufs=4) as outpool, \
             tc.tile_pool(name="p2psH", bufs=2, space="PSUM") as psH, \
             tc.tile_pool(name="p2psE", bufs=2, space="PSUM") as psE, \
             tc.tile_pool(name="p3ps", bufs=2, space="PSUM") as psT3:

            w1q = [None] * (E * 3)
            w2q = [None] * (E * 4)
            for kd in range(3):
                w1q[kd] = w1pre[kd]
            for k in range(4):
                w2q[k] = w2pre[k]
            for e in range(1, E):
                for kd in range(3):
                    w = w1pool.tile([128, 2, H], FP8, name=f"w1q{e}_{kd}",
                                    tag=f"w1q{e}_{kd}")
                    nc.sync.dma_start(out=w, in_=w1q_d.ap()[e, kd])
                    w1q[e * 3 + kd] = w
                for k in range(4):
                    w = w2pool.tile([128, 2, D], FP8, name=f"w2q{e}_{k}",
                                    tag=f"w2q{e}_{k}")
                    nc.sync.dma_start(out=w, in_=w2q_d.ap()[e, k])
                    w2q[e * 4 + k] = w

            g2b = be2b = None
            if not flags["ln2_id"]:
                g2b = cpool.tile([128, D], F32, name="g2b", tag="g2b")
                be2b = cpool.tile([128, D], F32, name="be2b", tag="be2b")
                nc.sync.dma_start(out=g2b, in_=_bcast_row(g2_d.ap(), 0, D))
                nc.sync.dma_start(out=be2b, in_=_bcast_row(be2_d.ap(), 0, D))
            b2sb = None
            if not flags["b2_zero"]:
                b2sb = cpool.tile([E, D], F32, name="b2sb", tag="b2sb")
                nc.sync.dma_start(out=b2sb, in_=b2_d.ap())
            cwsb = cpool.tile([128, KD, L], F32, name="cwsb", tag="cwsb")
            nc.sync.dma_start(out=cwsb, in_=cwj_d.ap())
            csb = cpool.tile([128, L], F32, name="csb", tag="csb")
            nc.sync.dma_start(out=csb, in_=_bcast_row(csum_d.ap(), 0, L))
            cbb = None
            if not flags["cb_zero"]:
                cbb = cpool.tile([128, L], F32, name="cbb", tag="cbb")
                nc.sync.dma_start(out=cbb, in_=_bcast_row(cb_d.ap(), 0, L))

            # csum-folded LN2 measured slower in the timeline sim than the
            # plain inline final; keep the plain path.
            fast_final = False

            def final_block(t):
                """residual LN2 + classifier for one finished token tile.

                Fast path: LN2's per-token (mu, rstd) commute with the
                classifier matmul: logits = rstd*(y@cw - mu*colsum(cw)), so
                the classifier runs on the UN-normalized residual and the
                LN fix-up is two tiny [128,L] ops -- the 768-wide LN apply
                disappears.
                """
                x = p3pool.tile([128, D], F32, name=f"x3{t}", tag="x3")
                if fast_final:
                    nc.vector.tensor_copy(out=x, in_=acc16[t])
                    stats = sm3.tile([128, 3, 6], F32, name=f"s3{t}", tag="s3")
                    for sg in range(3):
                        nc.vector.bn_stats(out=stats[:, sg, :],
                                           in_=acc16[t][:, sg * 256:(sg + 1) * 256])
                    mv = sm3.tile([128, 2], F32, name=f"mv3{t}", tag="mv3")
                    nc.vector.bn_aggr(out=mv, in_=stats)
                    rstd = newton_rsqrt(sm3, "L2", mv[:, 1:2], 2)
                    stg3 = p3pool.tile([128, KD, 128], F32, name=f"stg3{t}",
                                       tag="stg3")
                    for j in range(KD):
                        pt3 = psT3.tile([128, 128], F32, name=f"pt3{t}_{j}",
                                        tag="pt3")
                        nc.tensor.transpose(pt3, x[:, j * 128:(j + 1) * 128],
                                            ident)
                        nc.scalar.copy(out=stg3[:, j, :], in_=pt3)
                    pl = psT3.tile([128, L], F32, name=f"pl{t}", tag="pt3")
                    for j in range(KD):
                        nc.tensor.matmul(pl, stg3[:, j, :], cwsb[:, j, :],
                                         start=(j == 0), stop=(j == KD - 1))
                    mcs = sm3.tile([128, L], F32, name=f"mcs{t}", tag="mcs")
                    nc.vector.tensor_scalar(out=mcs, in0=csb,
                                            scalar1=mv[:, 0:1], scalar2=None,
                                            op0=OP.mult)
                    lt = outpool.tile([128, L], F32, name=f"lt{t}", tag="lt")
                    nc.vector.tensor_tensor(out=lt, in0=pl, in1=mcs,
                                            op=OP.subtract)
                    nc.vector.tensor_scalar_mul(out=lt, in0=lt, scalar1=rstd)
                    if cbb is not None:
                        nc.vector.tensor_tensor(out=lt, in0=lt, in1=cbb,
                                                op=OP.add)
                    nc.sync.dma_start(out=out_d.ap()[t * 128:(t + 1) * 128, :],
                                      in_=lt)
                    return
                if b2sb is not None:
                    ptc = psT3.tile([E, 128], F32, name=f"ptc{t}", tag="pt3")
                    nc.tensor.transpose(ptc, comb[t], ident)
                    cT = sm3.tile([E, 128], F32, name=f"cT{t}", tag="cT")
                    nc.scalar.copy(out=cT, in_=ptc)
                    pca = psT3.tile([128, 512], F32, name=f"pca{t}", tag="pca")
                    nc.tensor.matmul(pca, cT, b2sb[:, 0:512], start=True,
                                     stop=True)
                    nc.vector.scalar_tensor_tensor(
                        out=x[:, 0:512], in0=pca, scalar=SW2,
                        in1=acc16[t][:, 0:512], op0=OP.mult, op1=OP.add)
                    pcb = psT3.tile([128, 256], F32, name=f"pcb{t}", tag="pca")
                    nc.tensor.matmul(pcb, cT, b2sb[:, 512:768], start=True,
                                     stop=True)
                    nc.vector.scalar_tensor_tensor(
                        out=x[:, 512:768], in0=pcb, scalar=SW2,
                        in1=acc16[t][:, 512:768], op0=OP.mult, op1=OP.add)
                else:
                    nc.vector.tensor_copy(out=x, in_=acc16[t])

                stats = sm3.tile([128, 3, 6], F32, name=f"s3{t}", tag="s3")
                for sg in range(3):
                    nc.vector.bn_stats(out=stats[:, sg, :],
                                       in_=x[:, sg * 256:(sg + 1) * 256])
                mv = sm3.tile([128, 2], F32, name=f"mv3{t}", tag="mv3")
                nc.vector.bn_aggr(out=mv, in_=stats)
                rstd = newton_rsqrt(sm3, "L2", mv[:, 1:2], 2)
                nc.vector.tensor_scalar(out=x, in0=x, scalar1=mv[:, 0:1],
                                        scalar2=rstd, op0=OP.subtract,
                                        op1=OP.mult)
                if not flags["ln2_id"]:
                    nc.vector.tensor_tensor(out=x, in0=x, in1=g2b, op=OP.mult)
                    nc.vector.tensor_tensor(out=x, in0=x, in1=be2b, op=OP.add)
                stg3 = p3pool.tile([128, KD, 128], F32, name=f"stg3{t}",
                                   tag="stg3")
                for j in range(KD):
                    pt3 = psT3.tile([128, 128], F32, name=f"pt3{t}_{j}",
                                    tag="pt3")
                    nc.tensor.transpose(pt3, x[:, j * 128:(j + 1) * 128], ident)
                    nc.scalar.copy(out=stg3[:, j, :], in_=pt3)
                pl = psT3.tile([128, L], F32, name=f"pl{t}", tag="pt3")
                for j in range(KD):
                    nc.tensor.matmul(pl, stg3[:, j, :], cwsb[:, j, :],
                                     start=(j == 0), stop=(j == KD - 1))
                lt = outpool.tile([128, L], F32, name=f"lt{t}", tag="lt")
                if cbb is None:
                    nc.vector.tensor_copy(out=lt, in_=pl)
                else:
                    nc.vector.tensor_tensor(out=lt, in0=pl, in1=cbb, op=OP.add)
                nc.sync.dma_start(out=out_d.ap()[t * 128:(t + 1) * 128, :],
                                  in_=lt)

            for n in (range(NT) if "2" in PHASES else []):
                cs, ce = n * 128, (n + 1) * 128

                def mm1(e, k2):
                    """mm1 for h rows [512*k2, 512*k2+512) -> 4-plane psum."""
                    ph = psH.tile([128, 4, 128], F32, name=f"ph{n}_{e}_{k2}",
                                  tag="ph")
                    for m in range(4):
                        hs_ = (4 * k2 + m) * 128
                        for kd in range(3):
                            nc.tensor.matmul(
                                ph[:, m, :],
                                w1q[e * 3 + kd][:, :, hs_:hs_ + 128],
                                seq8[kd][:, :, cs:ce],
                                start=(kd == 0), stop=(kd == 2),
                                perf_mode=DR)
                    hq = hpool.tile([128, 4, 128], FP8, name=f"hq{n}_{e}_{k2}",
                                    tag="hq")
                    if flags["b1_zero"]:
                        nc.scalar.activation(out=hq, in_=ph, func=AF.Gelu,
                                             scale=1.0 / MOE_PSUM_SCALE)
                    else:
                        for m in range(4):
                            mh = 4 * k2 + m
                            nc.scalar.activation(
                                out=hq[:, m, :], in_=ph[:, m, :],
                                func=AF.Gelu, bias=b1sb[:, e, mh:mh + 1],
                                scale=1.0 / MOE_PSUM_SCALE)
                    return hq

                def mm2_and_combine(e, hqA, hqB):
                    pea = psE.tile([128, D], F32, name=f"pea{n}_{e}",
                                   tag="pea")
                    for kglob in range(4):
                        hq = hqA if kglob < 2 else hqB
                        kp = kglob % 2
                        st = hq[:, 2 * kp:2 * kp + 2, :]
                        nc.tensor.matmul(
                            pea[:, 0:512], st,
                            w2q[e * 4 + kglob][:, :, 0:512],
                            start=(kglob == 0), stop=(kglob == 3),
                            perf_mode=DR)
                        nc.tensor.matmul(
                            pea[:, 512:768], st,
                            w2q[e * 4 + kglob][:, :, 512:768],
                            start=(kglob == 0), stop=(kglob == 3),
                            perf_mode=DR)
                    c = comb[n][:, e:e + 1]
                    nc.vector.scalar_tensor_tensor(
                        out=acc16[n], in0=pea, scalar=c,
                        in1=acc16[n], op0=OP.mult, op1=OP.add)

                # expert-lag pipeline (depth 2): mm2 of expert e-2 runs
                # while the gelus of experts e-1/e are still in flight.
                # The previous tile's LN2+classifier is emitted mid-loop so
                # its transposes never wait on the LN2 DVE chain.
                import collections as _c
                lagq = _c.deque()
                for e in range(E):
                    hqA = mm1(e, 0)
                    hqB = mm1(e, 1)
                    if e == 3 and n > 0 and "3" in PHASES:
                        final_block(n - 1)
                    lagq.append((e, hqA, hqB))
                    if len(lagq) > 2:
                        mm2_and_combine(*lagq.popleft())
                while lagq:
                    mm2_and_combine(*lagq.popleft())
            if "2" in PHASES and "3" in PHASES:
                final_block(NT - 1)

    nc.compile()
    nc.finalize()
    return nc


def _get_nc(flags=None):
    if flags is None:
        flags = dict(FLAGS_DEFAULT)
    key = tuple(sorted(flags.items()))
    if key not in _CACHE:
        _CACHE[key] = _build(flags)
    return _CACHE[key]


def _flags_from_inputs(proj_b, ln1_g, ln1_b, gate_b, b1, b2, ln2_g, ln2_b,
                       cls_b):
    return dict(
        ln1_id=bool(np.all(np.asarray(ln1_g) == 1.0)
                    and np.all(np.asarray(ln1_b) == 0.0)),
        ln2_id=bool(np.all(np.asarray(ln2_g) == 1.0)
                    and np.all(np.asarray(ln2_b) == 0.0)),
        pb_zero=bool(np.all(np.asarray(proj_b) == 0.0)),
        b1_zero=bool(np.all(np.asarray(b1) == 0.0)),
        b2_zero=bool(np.all(np.asarray(b2) == 0.0)),
        cb_zero=bool(np.all(np.asarray(cls_b) == 0.0)),
    )


def _round_bits(a, nbits):
    """Round fp32 array to nbits explicit mantissa bits (round-to-nearest)."""
    u = np.ascontiguousarray(a, dtype=np.float32).view(np.uint32)
    shift = 23 - nbits
    half = np.uint32(1 << (shift - 1))
    mask = np.uint32(~((1 << shift) - 1) & 0xFFFFFFFF)
    return ((u + half) & mask).view(np.float32)


def _fp8(a):
    return np.ascontiguousarray(a, dtype=np.float32).astype(ml_dtypes.float8_e4m3)


def _pair(a):
    """[C, N] -> [C/256, 128, 2, N] pair-chunk layout for DoubleRow."""
    Cd, N = a.shape
    return np.ascontiguousarray(
        a.reshape(Cd // 256, 2, 128, N).transpose(0, 2, 1, 3))


def _prep_maps(hidden_states, proj_w, proj_b, ln1_g, ln1_b, gate_w, gate_b,
               w1, b1, w2, b2, ln2_g, ln2_b, cls_w, cls_b):
    f32 = np.float32
    pw = np.ascontiguousarray(proj_w, dtype=f32)
    pwh = _round_bits(pw, 11)
    pw8 = _pair(_fp8(pw * SW8))
    pwb = _pair(_fp8((pw - pwh) * SWB))
    shared = {
        "pwh": pwh,
        "pw8": pw8,
        "pwb": pwb,
        "pb": np.ascontiguousarray(proj_b, dtype=f32),
        "g1": np.ascontiguousarray(ln1_g, dtype=f32),
        "be1": np.ascontiguousarray(ln1_b, dtype=f32),
        "g2": np.ascontiguousarray(ln2_g, dtype=f32),
        "be2": np.ascontiguousarray(ln2_b, dtype=f32),
        "gw": np.ascontiguousarray(
            np.asarray(gate_w, dtype=f32).reshape(KD, 128, E).transpose(1, 0, 2)),
        "gb": np.ascontiguousarray(gate_b, dtype=f32),
        # w1 [E,D,H]: pair-chunks along D -> [E, 3, 128, 2, H] fp8 (*SW1)
        "w1q": np.ascontiguousarray(
            _fp8(np.asarray(w1, dtype=f32) * SW1).reshape(E, 3, 2, 128, H)
            .transpose(0, 1, 3, 2, 4)),
        # b1 [E,H] -> [128, E, KH]
        "b1": np.ascontiguousarray(
            np.asarray(b1, dtype=f32).reshape(E, KH, 128).transpose(2, 0, 1)),
        # w2 [E,H,D]: pair-chunks along H -> [E, 4, 128, 2, D] fp8 (*SW2)
        "w2q": np.ascontiguousarray(
            _fp8(np.asarray(w2, dtype=f32) * SW2).reshape(E, 4, 2, 128, D)
            .transpose(0, 1, 3, 2, 4)),
        "b2": np.ascontiguousarray(b2, dtype=f32),
        "cwj": np.ascontiguousarray(
            np.asarray(cls_w, dtype=f32).reshape(KD, 128, L).transpose(1, 0, 2)),
        "csum": np.ascontiguousarray(
            np.asarray(cls_w, dtype=f32).sum(axis=0)),
        "cb": np.ascontiguousarray(cls_b, dtype=f32),
    }
    hs = np.asarray(hidden_states, dtype=f32)
    per_core = B // NCORES
    maps = []
    for cid in range(NCORES):
        hT = np.ascontiguousarray(
            hs[cid * per_core:(cid + 1) * per_core].reshape(T, C).T)
        hTh = _round_bits(hT, 11)
        xa = _pair(_fp8((hT - hTh) * SA))       # [KP, 128, 2, T]
        x8 = _pair(_fp8(hT))                    # [KP, 128, 2, T]
        m = dict(shared)
        # tile-major hi part: [NT, 128, KC, 128]
        m["hh"] = np.ascontiguousarray(
            hTh.reshape(KC, 128, NT, 128).transpose(2, 1, 0, 3))
        # tile-major fp8 pieces: [NT, 128, KP, 2(piece), 2(plane), 128]
        xs = np.stack([xa, x8], axis=2)          # [KP, 128, 2pc, 2pl, T]
        m["xq"] = np.ascontiguousarray(
            xs.reshape(KP, 128, 2, 2, NT, 128).transpose(4, 1, 0, 2, 3, 5))
        maps.append(m)
    return maps


def kernel(**inputs) -> np.ndarray:
    flags = _flags_from_inputs(
        proj_b=inputs["proj_b"], ln1_g=inputs["ln1_g"], ln1_b=inputs["ln1_b"],
        gate_b=inputs["gate_b"], b1=inputs["b1"], b2=inputs["b2"],
        ln2_g=inputs["ln2_g"], ln2_b=inputs["ln2_b"], cls_b=inputs["cls_b"])
    nc = _get_nc(flags)
    maps = _prep_maps(**inputs)
    res = bass_utils.run_bass_kernel_spmd(nc, maps, core_ids=list(range(NCORES)))
    outs = [res.results[c]["out"] for c in range(NCORES)]
    full = np.concatenate(outs, axis=0).reshape(B, S, L)
    return full.astype(np.float32)
